# revision 40
# baseline (speedup 1.0000x reference)
"""Trainium2 Bass kernel for the NeuralODE (Tsit5, linear-in-t vector field) problem.

The reference integrates dy/dt = f(t) = t * w with Tsit5 on a fixed grid
ts[k] = k/T.  f is independent of y and linear in t, so the Tsit5 update
collapses exactly to y[k] = y0 + 0.5*ts[k]^2 * w (the order conditions give
sum(B)=1, sum(B*C)=1/2, and a 5th-order method integrates a linear f exactly).

Kernel strategy (per core, 8-way shard over the state dim D=8192 -> DS=1024):

  out[d, k] = (0.5*w[d]) * ts[k]^2 + y0[d]

  - state-major layout: partition = d (8 blocks of 128), free = k (4096).
    w/y0 become per-partition scalars, so each block is ONE fused DVE
    tensor_scalar (mult, add) op - fp16-in single-src ops run in 2x mode,
    ~2.2 us per [128, 4096] block, ~18 us total on DVE.
  - ts^2 broadcast: PE matmul ones(1,128)^T @ ts_bf16(1,512-chunk) -> PSUM,
    then ACT Square (PSUM -> fp16 SBUF).  This keeps the SDMA engines free
    for the output stream (a stride-0 broadcast DMA would share them) and
    PE/ACT are otherwise idle.  bf16 ts is plenty: total rel err ~6e-4
    against the fp32 reference (gate is 2e-2).
  - w/y0 per-partition columns arrive host-prelayouted as one [128, 16] f32
    input (wyc): a pure reshape/transpose of the shard, one tiny contiguous
    DMA instead of a 1024-descriptor gather.  The 0.5 scale is folded into
    the w column on device (one [128,8] DVE op).
  - output is written as float16 (rel-err budget 2e-2 >> fp16's ~5e-4),
    halving HBM write traffic: 8.39 MB/core instead of 16.78 MB.  The
    measured per-core HBM *write* wall under 8-core load is ~245 GB/s
    (~2 TB/s chip-wide), so the fp16 stream floor is ~34 us - which this
    kernel hits; everything else overlaps under it.
  - DRAM output is the transposed (DS, T) layout so each [128, T] block is
    one contiguous 1 MB DMA (per-partition 8 KB descriptors); the host
    gather is concat + transpose + f32 upcast (pure data movement/widening,
    all arithmetic stays on device).
  - the whole setup chain (wyc/ts/a2) is double-buffered (bufs=2 pool) so
    iteration i+1's PE/ACT refill overlaps iteration i's DVE consumption in
    the benchmark loop; block 0's DMA is split ragged (0.25/0.25/0.5 MB) so
    the write stream starts as early as possible.

Measured (8 cores concurrent, repeat-loop slope): ~34-36 us/iter vs the
59.8 us f32 baseline; rel err 6.4e-4.
"""

import numpy as np

_T = 4096
_D = 8192
_NCORES = 8
_DS = _D // _NCORES  # 1024 state elements per core
_P = 128
_NBLK = _DS // _P  # 8 partition blocks of the state dim
_CHUNK = 1024  # ts-broadcast chunk (PSUM tile free size)

_CACHE = {}


def _program(repeat=None, variant="full"):
    """Build (and cache) the Bass program. repeat=None emits the kernel body
    once; repeat=N wraps it in an on-device For_i loop (benchmarking only).

    variant:
      full   - the real kernel
      nodma  - compute only, output DMAs skipped (ablation)
      empty  - trivial body (loop-overhead measurement)
      tuning suffixes: "b<N>" big-pool bufs, "flat" no block-0 split,
      "s22" block-0 split [2048, 2048]
    """
    key = ("nc", repeat, variant)
    if key in _CACHE:
        return _CACHE[key]
    import concourse.bacc as bacc
    import concourse.mybir as mybir
    from concourse.tile import TileContext

    big_bufs = 8
    b0_splits = [1024, 1024, 2048]
    base = variant
    if "b6" in base:
        big_bufs = 6
    elif "b4" in base:
        big_bufs = 4
    if "flat" in base:
        b0_splits = [_T]
    elif "s22" in base:
        b0_splits = [2048, 2048]

    f32 = mybir.dt.float32
    f16 = mybir.dt.float16
    bf16 = mybir.dt.bfloat16
    nc = bacc.Bacc("TRN2", target_bir_lowering=False, debug=False)
    ts_d = nc.declare_dram_parameter("ts", [_T], f32, isOutput=False)
    # host-prelayouted per-partition columns: wyc[p, b] = y0[b*128+p],
    # wyc[p, 8+b] = w[b*128+p]  (pure reshape/transpose of the shard)
    wyc_d = nc.declare_dram_parameter("wyc", [_P, 2 * _NBLK], f32, isOutput=False)
    out_d = nc.declare_dram_parameter("out", [_DS, _T], f16, isOutput=True)

    def body(setup_pool, big_pool, psum_pool):
        if variant == "empty":
            tiny = setup_pool.tile([_P, 8], f32)
            nc.vector.memset(tiny[:], 0.0)
            return
        assert variant.startswith("full") or variant == "nodma"

        wyc = setup_pool.tile([_P, 2 * _NBLK], f32)
        nc.scalar.dma_start(out=wyc[:], in_=wyc_d[:])
        # wh = 0.5*w (absorbs the 0.5 of a = 0.5*ts^2)
        wh = setup_pool.tile([_P, _NBLK], f32)
        nc.vector.tensor_scalar_mul(wh[:], wyc[:, _NBLK : 2 * _NBLK], 0.5)

        ts_row = setup_pool.tile([1, _T], bf16)
        nc.gpsimd.dma_start(out=ts_row[:], in_=ts_d[:].unsqueeze(0))
        ones_row = setup_pool.tile([1, _P], bf16)
        nc.vector.memset(ones_row[:], 1.0)

        # a2[p, k] = ts[k]^2 for every partition p
        a2 = setup_pool.tile([_P, _T], f16)
        for h in range(_T // _CHUNK):
            sl = slice(h * _CHUNK, (h + 1) * _CHUNK)
            ps = psum_pool.tile([_P, _CHUNK], f32)
            for q in range(_CHUNK // 512):
                base = h * _CHUNK + q * 512
                nc.tensor.matmul(
                    ps[:, q * 512 : (q + 1) * 512],
                    ones_row[:],
                    ts_row[:, base : base + 512],
                    start=True,
                    stop=True,
                )
            nc.scalar.activation(
                a2[:, sl],
                ps[:],
                mybir.ActivationFunctionType.Square,
                bias=0.0,
                scale=1.0,
            )

        for b in range(_NBLK):
            splits = b0_splits if b == 0 else [_T]
            big = big_pool.tile([_P, _T], f16)
            off = 0
            for w_sz in splits:
                sl = slice(off, off + w_sz)
                nc.vector.tensor_scalar(
                    out=big[:, sl],
                    in0=a2[:, sl],
                    scalar1=wh[:, b : b + 1],
                    scalar2=wyc[:, b : b + 1],
                    op0=mybir.AluOpType.mult,
                    op1=mybir.AluOpType.add,
                )
                if variant != "nodma":
                    nc.sync.dma_start(
                        out=out_d[b * _P : (b + 1) * _P, sl], in_=big[:, sl]
                    )
                off += w_sz

    with TileContext(nc) as tc:
        with (
            tc.tile_pool(name="setup", bufs=2) as setup_pool,
            tc.tile_pool(name="big", bufs=big_bufs) as big_pool,
            tc.tile_pool(name="psum", bufs=3, space="PSUM") as psum_pool,
        ):
            if repeat is None:
                body(setup_pool, big_pool, psum_pool)
            else:
                with tc.For_i(0, repeat, 1):
                    body(setup_pool, big_pool, psum_pool)

    nc.compile()
    _CACHE[key] = nc
    return nc


def _run(ts, y0, W, trace=False, variant="full"):
    ts = np.ascontiguousarray(np.asarray(ts, dtype=np.float32))
    y0 = np.ascontiguousarray(np.asarray(y0, dtype=np.float32))
    W = np.ascontiguousarray(np.asarray(W, dtype=np.float32))
    assert ts.shape == (_T,) and y0.shape == (_D,) and W.shape == (1, _D)

    nc = _program(variant=variant)
    from concourse.bass_utils import run_bass_kernel_spmd

    in_maps = []
    for i in range(_NCORES):
        y0s = y0[i * _DS : (i + 1) * _DS]
        ws = W[0, i * _DS : (i + 1) * _DS]
        # per-partition column layout (reshape/transpose only, no math)
        wyc = np.ascontiguousarray(
            np.concatenate(
                [y0s.reshape(_NBLK, _P).T, ws.reshape(_NBLK, _P).T], axis=1
            )
        )
        in_maps.append({"ts": ts, "wyc": wyc})
    res = run_bass_kernel_spmd(nc, in_maps, list(range(_NCORES)), trace=trace)
    # gather: concat the state shards, undo the on-device transpose, widen fp16
    full = np.concatenate([res.results[i]["out"] for i in range(_NCORES)], axis=0)
    out = full.T.astype(np.float32, order="C")
    return out, res


def kernel(ts, y0, W):
    out, _ = _run(ts, y0, W, trace=False)
    return out


# revision 45
# speedup vs baseline: 1.0226x; 1.0226x over previous
"""Trainium2 Bass kernel for the NeuralODE (Tsit5, linear-in-t vector field) problem.

The reference integrates dy/dt = f(t) = t * w with Tsit5 on a fixed grid
ts[k] = k/T.  f is independent of y and linear in t, so the Tsit5 update
collapses exactly to y[k] = y0 + 0.5*ts[k]^2 * w (the order conditions give
sum(B)=1, sum(B*C)=1/2, and a 5th-order method integrates a linear f exactly).

Kernel strategy (per core, 8-way shard over the state dim D=8192 -> DS=1024):

  out[d, k] = (0.5*w[d]) * ts[k]^2 + y0[d]

  - state-major layout: partition = d (8 blocks of 128), free = k (4096).
    w/y0 become per-partition scalars, so each block is ONE fused DVE
    tensor_scalar (mult, add) op - fp16-in single-src ops run in 2x mode,
    ~2.2 us per [128, 4096] block, ~18 us total on DVE.
  - ts^2 broadcast: PE matmul ones(1,128)^T @ ts_bf16(1,512-chunk) -> PSUM,
    then ACT Square (PSUM -> fp16 SBUF).  This keeps the SDMA engines free
    for the output stream (a stride-0 broadcast DMA would share them) and
    PE/ACT are otherwise idle.  bf16 ts is plenty: total rel err ~6e-4
    against the fp32 reference (gate is 2e-2).
  - w/y0 per-partition columns arrive host-prelayouted as one [128, 16] f32
    input (wyc): a pure reshape/transpose of the shard, one tiny contiguous
    DMA instead of a 1024-descriptor gather.  The 0.5 scale is folded into
    the w column on device (one [128,8] DVE op).
  - output is written as float16 (rel-err budget 2e-2 >> fp16's ~5e-4),
    halving HBM write traffic: 8.39 MB/core instead of 16.78 MB.  The
    measured per-core HBM *write* wall under 8-core load is ~245 GB/s
    (~2 TB/s chip-wide), so the fp16 stream floor is ~34 us - which this
    kernel hits; everything else overlaps under it.
  - DRAM output is the transposed (DS, T) layout so each [128, T] block is
    one contiguous 1 MB DMA (per-partition 8 KB descriptors); the host
    gather is concat + transpose + f32 upcast (pure data movement/widening,
    all arithmetic stays on device).
  - the whole setup chain (wyc/ts/a2) is double-buffered (bufs=2 pool) so
    iteration i+1's PE/ACT refill overlaps iteration i's DVE consumption in
    the benchmark loop; block 0's DMA is split ragged (0.25/0.25/0.5 MB) so
    the write stream starts as early as possible.

Measured (8 cores concurrent, repeat-loop slope): ~34-36 us/iter vs the
59.8 us f32 baseline; rel err 6.4e-4.
"""

import numpy as np

_T = 4096
_D = 8192
_NCORES = 8
_DS = _D // _NCORES  # 1024 state elements per core
_P = 128
_NBLK = _DS // _P  # 8 partition blocks of the state dim
_CHUNK = 1024  # ts-broadcast chunk (PSUM tile free size)

_CACHE = {}


def _program(repeat=None, variant="full"):
    """Build (and cache) the Bass program. repeat=None emits the kernel body
    once; repeat=N wraps it in an on-device For_i loop (benchmarking only).

    variant:
      full   - the real kernel
      nodma  - compute only, output DMAs skipped (ablation)
      empty  - trivial body (loop-overhead measurement)
      tuning suffixes: "b<N>" big-pool bufs, "flat" no block-0 split,
      "s22" block-0 split [2048, 2048]
    """
    key = ("nc", repeat, variant)
    if key in _CACHE:
        return _CACHE[key]
    import concourse.bacc as bacc
    import concourse.mybir as mybir
    from concourse.tile import TileContext

    big_bufs = 8
    b0_splits = [1024, 1024, 2048]
    base = variant
    if "b6" in base:
        big_bufs = 6
    elif "b4" in base:
        big_bufs = 4
    elif "b10" in base:
        big_bufs = 10
    elif "b12" in base:
        big_bufs = 12
    if "flat" in base:
        b0_splits = [_T]
    elif "s22" in base:
        b0_splits = [2048, 2048]

    f32 = mybir.dt.float32
    f16 = mybir.dt.float16
    bf16 = mybir.dt.bfloat16
    nc = bacc.Bacc("TRN2", target_bir_lowering=False, debug=False)
    ts_d = nc.declare_dram_parameter("ts", [_T], f32, isOutput=False)
    # host-prelayouted per-partition columns: wyc[p, b] = y0[b*128+p],
    # wyc[p, 8+b] = w[b*128+p]  (pure reshape/transpose of the shard)
    wyc_d = nc.declare_dram_parameter("wyc", [_P, 2 * _NBLK], f32, isOutput=False)
    out_d = nc.declare_dram_parameter("out", [_DS, _T], f16, isOutput=True)

    def body(setup_pool, big_pool, psum_pool):
        if variant == "empty":
            tiny = setup_pool.tile([_P, 8], f32)
            nc.vector.memset(tiny[:], 0.0)
            return
        assert variant.startswith("full") or variant == "nodma"

        wyc = setup_pool.tile([_P, 2 * _NBLK], f32)
        nc.scalar.dma_start(out=wyc[:], in_=wyc_d[:])
        # wh = 0.5*w (absorbs the 0.5 of a = 0.5*ts^2)
        wh = setup_pool.tile([_P, _NBLK], f32)
        nc.vector.tensor_scalar_mul(wh[:], wyc[:, _NBLK : 2 * _NBLK], 0.5)

        ts_row = setup_pool.tile([1, _T], bf16)
        nc.gpsimd.dma_start(out=ts_row[:], in_=ts_d[:].unsqueeze(0))
        ones_row = setup_pool.tile([1, _P], bf16)
        nc.vector.memset(ones_row[:], 1.0)

        # a2[p, k] = ts[k]^2 for every partition p
        a2 = setup_pool.tile([_P, _T], f16)
        for h in range(_T // _CHUNK):
            sl = slice(h * _CHUNK, (h + 1) * _CHUNK)
            ps = psum_pool.tile([_P, _CHUNK], f32)
            for q in range(_CHUNK // 512):
                base = h * _CHUNK + q * 512
                nc.tensor.matmul(
                    ps[:, q * 512 : (q + 1) * 512],
                    ones_row[:],
                    ts_row[:, base : base + 512],
                    start=True,
                    stop=True,
                )
            nc.scalar.activation(
                a2[:, sl],
                ps[:],
                mybir.ActivationFunctionType.Square,
                bias=0.0,
                scale=1.0,
            )

        for b in range(_NBLK):
            splits = b0_splits if b == 0 else [_T]
            big = big_pool.tile([_P, _T], f16)
            off = 0
            for w_sz in splits:
                sl = slice(off, off + w_sz)
                nc.vector.tensor_scalar(
                    out=big[:, sl],
                    in0=a2[:, sl],
                    scalar1=wh[:, b : b + 1],
                    scalar2=wyc[:, b : b + 1],
                    op0=mybir.AluOpType.mult,
                    op1=mybir.AluOpType.add,
                )
                if variant != "nodma":
                    nc.sync.dma_start(
                        out=out_d[b * _P : (b + 1) * _P, sl], in_=big[:, sl]
                    )
                off += w_sz

    with TileContext(nc) as tc:
        with (
            tc.tile_pool(name="setup", bufs=2) as setup_pool,
            tc.tile_pool(name="big", bufs=big_bufs) as big_pool,
            tc.tile_pool(name="psum", bufs=3, space="PSUM") as psum_pool,
        ):
            if repeat is None:
                body(setup_pool, big_pool, psum_pool)
            else:
                with tc.For_i(0, repeat, 1):
                    body(setup_pool, big_pool, psum_pool)

    nc.compile()
    _CACHE[key] = nc
    return nc


def _run(ts, y0, W, trace=False, variant="full"):
    ts = np.ascontiguousarray(np.asarray(ts, dtype=np.float32))
    y0 = np.ascontiguousarray(np.asarray(y0, dtype=np.float32))
    W = np.ascontiguousarray(np.asarray(W, dtype=np.float32))
    assert ts.shape == (_T,) and y0.shape == (_D,) and W.shape == (1, _D)

    nc = _program(variant=variant)
    from concourse.bass_utils import run_bass_kernel_spmd

    in_maps = []
    for i in range(_NCORES):
        y0s = y0[i * _DS : (i + 1) * _DS]
        ws = W[0, i * _DS : (i + 1) * _DS]
        # per-partition column layout (reshape/transpose only, no math)
        wyc = np.ascontiguousarray(
            np.concatenate(
                [y0s.reshape(_NBLK, _P).T, ws.reshape(_NBLK, _P).T], axis=1
            )
        )
        in_maps.append({"ts": ts, "wyc": wyc})
    res = run_bass_kernel_spmd(nc, in_maps, list(range(_NCORES)), trace=trace)
    # gather: concat the state shards, undo the on-device transpose, widen fp16
    full = np.concatenate([res.results[i]["out"] for i in range(_NCORES)], axis=0)
    out = full.T.astype(np.float32, order="C")
    return out, res


def kernel(ts, y0, W):
    out, _ = _run(ts, y0, W, trace=False)
    return out
